# revision 17
# baseline (speedup 1.0000x reference)
"""MultiHeadAttention (B=4,T=2048,D=2048,NQ=16,NK=8,H=128) on 8 trn2 cores.

Sharding: core c -> batch b=c//2, half=c%2. Each core computes the partial
output for batch b restricted to q-heads [half*8, half*8+8) (kv-heads
[half*4, half*4+4)); host sums the two partials per batch (o_proj
contraction over heads is split across the core pair).

v3: V projection computed with 512-col matmuls (4 kv heads batched in the
free dim) eliminating 768 tiny matmuls; o_proj runs in fp8e4 DoubleRow
(2 heads' contraction per pass); RMSNorm gain broadcast moved to gpsimd;
o_proj psum->sbuf staging moved to gpsimd with the fp8 weight prescale
folded in; startup DMA order tuned (per-pair q weights, per-chunk rope
tables) so the first matmul issues ~8us in.
"""
import numpy as np
import concourse.bass as bass
import concourse.tile as tile
from concourse import bacc, mybir
from concourse import bass_utils

B, T, D = 4, 2048, 2048
NQ, NK, H = 16, 8, 128
NH, NKV = 8, 4          # per-core q heads / kv heads
THETA = 10000.0
EPS = 1e-6
TCH = 512               # chunk of T for projections / attention q blocks
NCH = T // TCH
NDK = D // 128
NQC = T // TCH
WO_SCALE = 256.0        # fp8 o_proj weight prescale

f32 = mybir.dt.float32
bf16 = mybir.dt.bfloat16
f8 = mybir.dt.float8e4
npbf16 = mybir.dt.np(bf16)
npf8 = mybir.dt.np(f8)
AF = mybir.ActivationFunctionType
DR = mybir.MatmulPerfMode.DoubleRow

TRACE = False
LAST_EXEC_NS = None
_CACHE = {}


def _install_hook():
    import contextlib, ctypes, sys, types
    if "antenv.axon_hooks" in sys.modules:
        return
    lib = ctypes.CDLL("/opt/axon/libaxon_pjrt.so")
    lib.axon_start_nrt_profile.argtypes = [ctypes.POINTER(ctypes.c_int64), ctypes.c_size_t]
    lib.axon_start_nrt_profile.restype = ctypes.c_int64
    lib.axon_stop_nrt_profile.argtypes = [ctypes.c_char_p]
    lib.axon_stop_nrt_profile.restype = ctypes.c_int64

    @contextlib.contextmanager
    def _hook(output_dir, device_ids):
        import jax
        jax.devices()
        ids = (ctypes.c_int64 * len(device_ids))(*device_ids) if device_ids else None
        rc = lib.axon_start_nrt_profile(ids, len(device_ids) if device_ids else 0)
        if rc != 0:
            raise RuntimeError(f"axon_start_nrt_profile rc={rc}")
        try:
            yield
        finally:
            n = lib.axon_stop_nrt_profile(str(output_dir).encode())
            if n < 0:
                raise RuntimeError(f"axon_stop_nrt_profile rc={n}")

    mod = types.ModuleType("antenv.axon_hooks")
    mod.get_axon_ntff_profile_hook = lambda: _hook
    mod.set_axon_ntff_profile_hook = lambda h: None
    sys.modules["antenv.axon_hooks"] = mod
    bass_utils.upload_artifacts = lambda tmpdir: "local://" + str(tmpdir)


def _build():
    nc = bacc.Bacc("TRN2", target_bir_lowering=False, debug=False, num_devices=8)
    xt_ap = nc.dram_tensor("xt", [D, T], bf16, kind="ExternalInput").ap()
    wq_aps = [nc.dram_tensor(f"wq{p}", [128, 2 * NDK * 128], bf16,
                             kind="ExternalInput").ap() for p in range(4)]
    wk_ap = nc.dram_tensor("wk", [128, NKV * NDK * 128], bf16, kind="ExternalInput").ap()
    wv_ap = nc.dram_tensor("wv", [128, NDK * NKV * 128], bf16, kind="ExternalInput").ap()
    wo_ap = nc.dram_tensor("wo", [128, NH * D], f8, kind="ExternalInput").ap()
    cs_ap = nc.dram_tensor("cs", [128, T], bf16, kind="ExternalInput").ap()
    sn_ap = nc.dram_tensor("sn", [128, T], bf16, kind="ExternalInput").ap()
    qsc_ap = nc.dram_tensor("qsc", [128, 1], f32, kind="ExternalInput").ap()
    ksc_ap = nc.dram_tensor("ksc", [128, 1], f32, kind="ExternalInput").ap()
    cm_ap = nc.dram_tensor("cm", [128, 128], bf16, kind="ExternalInput").ap()
    rm_ap = nc.dram_tensor("rmat", [128, 128], bf16, kind="ExternalInput").ap()
    out_ap = nc.dram_tensor("out", [T, D], bf16, kind="ExternalOutput").ap()

    with tile.TileContext(nc) as tc:
        with tc.tile_pool(name="mp", bufs=1) as mp, \
             tc.tile_pool(name="pp", bufs=1, space="PSUM") as pp:
            # ---- persistent tiles; DMA queue order is startup-critical ----
            qsc_t = mp.tile([128, 1], f32)
            nc.sync.dma_start(qsc_t[:], qsc_ap[:])
            ksc_t = mp.tile([128, 1], f32)
            nc.sync.dma_start(ksc_t[:], ksc_ap[:])
            cm_t = mp.tile([128, 128], bf16)
            nc.sync.dma_start(cm_t[:], cm_ap[:])
            rmat_t = mp.tile([128, 128], bf16)
            nc.sync.dma_start(rmat_t[:], rm_ap[:])

            wq_ts = []
            for p in range(4):
                wt = mp.tile([128, 2 * NDK * 128], bf16, name=f"wq{p}")
                wq_ts.append(wt)
            xt_r = xt_ap.rearrange("(a p) t -> p a t", p=128)

            def load_xh(pool, ch):
                t0 = ch * TCH
                xh0 = pool.tile([128, 8 * TCH], bf16, tag="xh0", bufs=2, name="xh0")
                nc.sync.dma_start(
                    xh0[:].rearrange("p (a t) -> p a t", a=8),
                    xt_r[:, 0:8, t0:t0 + TCH])
                xh1 = pool.tile([128, 8 * TCH], bf16, tag="xh1", bufs=2, name="xh1")
                nc.sync.dma_start(
                    xh1[:].rearrange("p (a t) -> p a t", a=8),
                    xt_r[:, 8:16, t0:t0 + TCH])
                return xh0, xh1

            # q weights for heads 0-1 first, then x chunk 0, then the rest.
            nc.sync.dma_start(wq_ts[0][:], wq_aps[0][:])

            eps_t = mp.tile([1, 1], f32)
            nc.vector.memset(eps_t[:], EPS)
            ones_col_b = mp.tile([128, 1], bf16)
            nc.vector.memset(ones_col_b[:], 1.0)

            qT = mp.tile([128, NH * T], bf16)     # 32KB/part
            kT = mp.tile([128, NKV * T], bf16)    # 16KB
            vT = mp.tile([128, NKV * T], bf16)    # 16KB; cols = btile*512+kv*128+h

            rows = pp.tile([128, 512], f32, tag="a7")

            cs_ts, sn_ts = [], []
            wo_t = mp.tile([128, NH * D], f8)

            def drain_group(pool, accs, sc_t, ch, dsts):
                """accs: psum [128,512] f32 -> RMSNorm*(gain) + RoPE -> dsts bf16."""
                n = len(accs)
                for i in range(n):
                    sq = pool.tile([128, TCH], bf16, tag="sq", bufs=2, name=f"sq{i}")
                    nc.scalar.activation(sq[:], accs[i], AF.Square)
                    row = pp.tile([1, 512], f32, tag="a4", name=f"row{i}")
                    nc.tensor.matmul(row[:], ones_col_b[:], sq[:], start=True, stop=True)
                    rinv = pool.tile([1, TCH], f32, tag="rinv", bufs=2, name=f"rinv{i}")
                    nc.vector.reciprocal_approx_fast(rinv[:], row[:])
                    rstd = pool.tile([1, TCH], bf16, tag="rstd", bufs=2, name=f"rstd{i}")
                    nc.scalar.activation(rstd[:], rinv[:], AF.Sqrt)
                    bc = pool.tile([128, TCH], bf16, tag="bc", bufs=2, name=f"bc{i}")
                    nc.gpsimd.partition_broadcast(bc[:], rstd[:])
                    # qn0 = acc * gain (per-partition); rstd applied after rope
                    # (valid: rstd is per-column, invariant under the half-swap)
                    qn = pool.tile([128, TCH], bf16, tag="qn", bufs=2, name=f"qn{i}")
                    nc.scalar.activation(qn[:], accs[i], AF.Copy, scale=sc_t[:])
                    qsw = pp.tile([128, 512], f32, tag=f"a{5 + i % 2}", name=f"qsw{i}")
                    nc.tensor.matmul(qsw[:], rmat_t[:], qn[:], start=True, stop=True)
                    ta = pool.tile([128, TCH], bf16, tag="ta", bufs=2, name=f"ta{i}")
                    nc.vector.tensor_mul(ta[:], qn[:], cs_ts[ch][:])
                    tb = pool.tile([128, TCH], bf16, tag="tb", bufs=2, name=f"tb{i}")
                    nc.vector.tensor_mul(tb[:], qsw[:], sn_ts[ch][:])
                    rs = pool.tile([128, TCH], bf16, tag="rs", bufs=2, name=f"rs{i}")
                    nc.vector.tensor_add(rs[:], ta[:], tb[:])
                    nc.vector.tensor_mul(dsts[i], rs[:], bc[:])

            # ---- phase 1: q/k/v projection, norm+rope, all SBUF-resident ----
            with tc.tile_pool(name="pj", bufs=1) as pj:
                acc_roll = [0]

                def chain(w_t, whead, xh0, xh1):
                    """Sequential 16-dk matmul chain into one rolling psum bank."""
                    acc = pp.tile([128, 512], f32, tag=f"a{acc_roll[0] % 4}", name="acc")
                    acc_roll[0] += 1
                    for dk in range(NDK):
                        xh = xh0 if dk < 8 else xh1
                        nc.tensor.matmul(
                            acc[:],
                            w_t[:, (whead * NDK + dk) * 128:(whead * NDK + dk + 1) * 128],
                            xh[:, (dk % 8) * TCH:(dk % 8 + 1) * TCH],
                            start=(dk == 0), stop=(dk == NDK - 1))
                    return acc

                xhs = load_xh(pj, 0)
                # remaining weight/table loads (sync queue, after chunk-0 x)
                for p in range(1, 4):
                    nc.sync.dma_start(wq_ts[p][:], wq_aps[p][:])
                wk_t = pj.tile([128, NKV * NDK * 128], bf16)
                for j in range(2):
                    s = NKV * NDK * 128 // 2
                    nc.sync.dma_start(wk_t[:, j * s:(j + 1) * s], wk_ap[:, j * s:(j + 1) * s])
                wv_t = pj.tile([128, NDK * NKV * 128], bf16)
                for j in range(2):
                    s = NDK * NKV * 128 // 2
                    nc.sync.dma_start(wv_t[:, j * s:(j + 1) * s], wv_ap[:, j * s:(j + 1) * s])
                for ch in range(NCH):
                    t0 = ch * TCH
                    cst = mp.tile([128, TCH], bf16, name=f"cs{ch}")
                    nc.sync.dma_start(cst[:], cs_ap[:, t0:t0 + TCH])
                    snt = mp.tile([128, TCH], bf16, name=f"sn{ch}")
                    nc.sync.dma_start(snt[:], sn_ap[:, t0:t0 + TCH])
                    cs_ts.append(cst)
                    sn_ts.append(snt)
                for ch in range(NCH):
                    t0 = ch * TCH
                    xh0, xh1 = xhs
                    # prefetch next chunk's x before this chunk's drains/V
                    if ch + 1 < NCH:
                        xhs = load_xh(pj, ch + 1)
                    # q heads, two groups of 4
                    for g in range(2):
                        accs = [chain(wq_ts[(g * 4 + i) // 2], (g * 4 + i) % 2, xh0, xh1)
                                for i in range(4)]
                        drain_group(pj, [a[:] for a in accs], qsc_t, ch,
                                    [qT[:, (g * 4 + i) * T + t0:(g * 4 + i) * T + t0 + TCH]
                                     for i in range(4)])
                    # k heads
                    accs = [chain(wk_t, i, xh0, xh1) for i in range(NKV)]
                    drain_group(pj, [a[:] for a in accs], ksc_t, ch,
                                [kT[:, kv * T + t0:kv * T + t0 + TCH] for kv in range(NKV)])
                    # v: [t, (kv h)] orientation via 512-col matmuls
                    # (lhsT = x t-block, rhs = all 4 kv heads' w columns)
                    for tb_ in range(4):
                        vps = pp.tile([128, 512], f32, tag=f"a{acc_roll[0] % 4}", name="vps")
                        acc_roll[0] += 1
                        for dk in range(NDK):
                            xh = xh0 if dk < 8 else xh1
                            nc.tensor.matmul(
                                vps[:],
                                xh[:, (dk % 8) * TCH + tb_ * 128:(dk % 8) * TCH + (tb_ + 1) * 128],
                                wv_t[:, dk * 512:(dk + 1) * 512],
                                start=(dk == 0), stop=(dk == NDK - 1),
                                skip_group_check=True)
                        nc.vector.tensor_copy(
                            vT[:, (ch * 4 + tb_) * 512:(ch * 4 + tb_ + 1) * 512], vps[:])
                    if ch == 0:
                        # o_proj weights after chunk-0 traffic (needed in phase 2)
                        for j in range(4):
                            s = NH * D // 4
                            nc.sync.dma_start(wo_t[:, j * s:(j + 1) * s],
                                              wo_ap[:, j * s:(j + 1) * s])

            # ---- phase 2: attention + fused o_proj (fp8 DoubleRow) ----
            # o_proj units of chunk qi-1 are interleaved between attention
            # heads of chunk qi: their matmuls fill the PE stalls where PV
            # waits on exp, and they use no table-based ACT functions.
            with tc.tile_pool(name="op", bufs=1) as op:
                wo_r = wo_t[:].rearrange("p (h d) -> p h d", h=NH)
                out_r = out_ap.rearrange("(a p) d -> p a d", p=128)
                pend = []

                def mk_head(qi, h, attn_sb, nkj, q0):
                    def f():
                        kv = h // 2
                        o_ps = pp.tile([128, 512], f32, tag=["a3", "a5", "a6"][h % 3], name="ops")
                        acc_sb = op.tile([128, TCH], bf16, tag="accsb", bufs=3, name="accsb")

                        def emit_s(kj):
                            m = kj - 4 * qi
                            lo = 128 * m if m > 0 else 0
                            s_ps = pp.tile([128, 512], f32, tag=f"a{kj % 3}", name="sps")
                            nc.tensor.matmul(
                                s_ps[:, lo:512],
                                kT[:, kv * T + kj * 128:kv * T + (kj + 1) * 128],
                                qT[:, h * T + q0 + lo:h * T + q0 + TCH],
                                start=True, stop=True)
                            pt = op.tile([128, TCH], bf16, tag="pt", bufs=6, name="pt")
                            return s_ps, pt, lo, m, kj

                        def emit_drain(s_ps, pt, lo, m, kj):
                            nc.scalar.activation(pt[:, lo:512], s_ps[:, lo:512], AF.Exp)
                            if m >= 0:
                                nc.vector.tensor_mul(pt[:, lo:lo + 128],
                                                     pt[:, lo:lo + 128], cm_t[:])
                            if kj == 0:
                                nc.vector.tensor_copy(acc_sb[:], pt[:])
                            else:
                                nc.vector.tensor_add(acc_sb[:, lo:512],
                                                     acc_sb[:, lo:512], pt[:, lo:512])
                            nc.tensor.matmul(
                                o_ps[:, lo:512],
                                vT[:, kj * 512 + kv * 128:kj * 512 + (kv + 1) * 128],
                                pt[:, lo:512],
                                start=(kj == 0), stop=(kj == nkj - 1),
                                skip_group_check=True)

                        prev = None
                        for kj in range(nkj):
                            cur = emit_s(kj)
                            if prev is not None:
                                emit_drain(*prev)
                            prev = cur
                        emit_drain(*prev)

                        # softmax denominator for this (h, qi).
                        # NB: reciprocal_approx_fast corrupts results when its
                        # input sits at a nonzero base partition -> keep row 0.
                        nc.tensor.matmul(rows[0:1, :], ones_col_b[:],
                                         acc_sb[:], start=True, stop=True)
                        rden = op.tile([1, TCH], f32, tag="rden", bufs=3, name="rden")
                        nc.vector.reciprocal_approx_fast(rden[:], rows[0:1, :])
                        rbc = op.tile([128, TCH], f32, tag="rbc", bufs=3, name="rbc")
                        nc.gpsimd.partition_broadcast(rbc[:], rden[:])
                        nc.vector.tensor_mul(
                            attn_sb[:, h * TCH:(h + 1) * TCH], o_ps[:], rbc[:])
                    return f

                def mk_oproj(qi, dc, attn_r):
                    def f():
                        stg4 = op.tile([128, 4 * 512], bf16, tag="ostg", bufs=2, name="ostg")
                        for ti in range(4):
                            ops2 = pp.tile([128, 512], f32, tag="a4", name="ops2")
                            for g in range(4):
                                nc.tensor.matmul(
                                    ops2[:],
                                    attn_r[:, 2 * g:2 * g + 2, ti * 128:(ti + 1) * 128],
                                    wo_r[:, 2 * g:2 * g + 2, dc * 512:(dc + 1) * 512],
                                    start=(g == 0), stop=(g == 3),
                                    perf_mode=DR)
                            # psum -> sbuf with the fp8 weight prescale undone
                            if ti % 2 == 0:
                                nc.vector.tensor_scalar_mul(
                                    stg4[:, ti * 512:(ti + 1) * 512], ops2[:],
                                    1.0 / WO_SCALE)
                            else:
                                nc.scalar.activation(stg4[:, ti * 512:(ti + 1) * 512],
                                                     ops2[:], AF.Copy, scale=1.0 / WO_SCALE)
                        nc.sync.dma_start(
                            out_r[:, qi * 4:qi * 4 + 4, dc * 512:(dc + 1) * 512],
                            stg4[:].rearrange("p (a d) -> p a d", a=4))
                    return f

                for qi in range(NQC):
                    q0 = qi * TCH
                    attn_sb = op.tile([128, NH * TCH], f8, tag="attn", bufs=2, name="attn")
                    attn_r = attn_sb[:].rearrange("p (h t) -> p h t", h=NH)
                    nkj = 4 * qi + 4
                    for h in range(NH):
                        mk_head(qi, h, attn_sb, nkj, q0)()
                        if pend and h % 2 == 1:
                            pend.pop(0)()
                    pend = [mk_oproj(qi, dc, attn_r) for dc in range(4)]
                for u in pend:
                    u()

    nc.compile()
    return nc


def _pack(w):
    """(nh, D, H) -> (128, nh*NDK*128): col block (h*NDK+dk)*128 = w[h, dk*128:+128, :]."""
    nh = w.shape[0]
    a = w.reshape(nh, NDK, 128, H).transpose(2, 0, 1, 3)
    return np.ascontiguousarray(a.reshape(128, nh * NDK * H)).astype(npbf16)


def _pack_v(w):
    """(NKV, D, H) -> (128, NDK*NKV*128): col dk*512+kv*128+h = w[kv, dk*128+p, h]."""
    a = w.reshape(NKV, NDK, 128, H).transpose(2, 1, 0, 3)
    return np.ascontiguousarray(a.reshape(128, NDK * NKV * H)).astype(npbf16)


def _numpy_ref(x, mask, position, qp, kvp, op, qns, kns):
    def rms(v, s):
        var = (v * v).mean(-1, keepdims=True)
        return v / np.sqrt(var + EPS) * (1.0 + s)

    def rope(v, pos):
        ts = THETA ** (np.arange(64, dtype=np.float32) * 2.0 / H)
        ang = pos.astype(np.float32)[:, :, None, None] / ts
        sn, cs = np.sin(ang), np.cos(ang)
        x1, x2 = v[..., :64], v[..., 64:]
        return np.concatenate([x1 * cs - x2 * sn, x2 * cs + x1 * sn], -1)

    q = np.einsum('BTD,NDH->BTNH', x, qp)
    k = np.einsum('BTD,KDH->BTKH', x, kvp[0])
    v = np.einsum('BTD,KDH->BTKH', x, kvp[1])
    q = rope(rms(q, qns), position) * (H ** -0.5)
    k = rope(rms(k, kns), position)
    q = q.transpose(0, 2, 1, 3)
    k = np.repeat(k.transpose(0, 2, 1, 3), NQ // NK, 1)
    v = np.repeat(v.transpose(0, 2, 1, 3), NQ // NK, 1)
    s = np.einsum('BHtD,BHTD->BHtT', q, k) / np.sqrt(np.float32(H))
    s = np.where(mask[:, None], s, np.float32(-2.3819763e+38))
    s = s - s.max(-1, keepdims=True)
    w = np.exp(s)
    w /= w.sum(-1, keepdims=True)
    o = np.einsum('BHtT,BHTD->BHtD', w, v)
    return np.einsum('BNTH,NHD->BTD', o, op).astype(np.float32)


def kernel(**inputs):
    global LAST_EXEC_NS
    x = np.asarray(inputs["x"], np.float32)
    mask = np.asarray(inputs["mask"])
    position = np.asarray(inputs["position"])
    qp = np.asarray(inputs["q_proj"], np.float32)
    kvp = np.asarray(inputs["kv_proj"], np.float32)
    op = np.asarray(inputs["o_proj"], np.float32)
    qns = np.asarray(inputs["q_norm_scale"], np.float32)
    kns = np.asarray(inputs["k_norm_scale"], np.float32)

    tril = np.tril(np.ones((T, T), bool))
    if mask.shape != (B, T, T) or not all(np.array_equal(mask[b], tril) for b in range(B)):
        return _numpy_ref(x, mask, position, qp, kvp, op, qns, kns)

    if "nc" not in _CACHE:
        _CACHE["nc"] = _build()
    nc = _CACHE["nc"]

    halves = []
    for half in range(2):
        wq_full = _pack(qp[half * NH:(half + 1) * NH])
        wqs = [np.ascontiguousarray(wq_full[:, p * 2 * NDK * 128:(p + 1) * 2 * NDK * 128])
               for p in range(4)]
        halves.append((
            wqs,
            _pack(kvp[0, half * NKV:(half + 1) * NKV]),
            _pack_v(kvp[1, half * NKV:(half + 1) * NKV]),
            np.ascontiguousarray(
                (op[half * NH:(half + 1) * NH] * WO_SCALE)
                .transpose(1, 0, 2).reshape(128, NH * D)
            ).astype(npf8),
        ))
    qsc = ((1.0 + qns) / np.sqrt(H)).reshape(128, 1).astype(np.float32)
    ksc = ((1.0 + kns) * np.sqrt(H)).reshape(128, 1).astype(np.float32)
    ts = THETA ** (np.arange(64, dtype=np.float64) * 2.0 / H)
    pidx = np.arange(128)[:, None]
    fidx = np.arange(128)[None, :]
    cm = (fidx >= pidx).astype(npbf16)
    rmat = np.zeros((128, 128), np.float32)
    rmat[(np.arange(128) + 64) % 128, np.arange(128)] = 1.0
    rmat = rmat.astype(npbf16)

    in_maps = []
    for c in range(8):
        b, half = c // 2, c % 2
        wqs, wk, wv, wo = halves[half]
        ang = position[b].astype(np.float64)[None, :] / ts[:, None]
        sn = np.sin(ang).astype(np.float32)
        cs = np.cos(ang).astype(np.float32)
        im = {
            "xt": np.ascontiguousarray(x[b].T).astype(npbf16),
            "wk": wk, "wv": wv, "wo": wo,
            "cs": np.ascontiguousarray(np.concatenate([cs, cs], 0)).astype(npbf16),
            "sn": np.ascontiguousarray(np.concatenate([-sn, sn], 0)).astype(npbf16),
            "qsc": qsc, "ksc": ksc, "cm": cm, "rmat": rmat,
        }
        for p in range(4):
            im[f"wq{p}"] = wqs[p]
        in_maps.append(im)

    if TRACE:
        _install_hook()
    last_err = None
    for _ in range(3):
        try:
            res = bass_utils.run_bass_kernel_spmd(nc, in_maps, list(range(8)), trace=TRACE)
            break
        except Exception as e:  # transient NRT device wedge
            last_err = e
    else:
        raise last_err
    LAST_EXEC_NS = getattr(res, "exec_time_ns", None)

    out = np.empty((B, T, D), np.float32)
    for b in range(B):
        out[b] = (res.results[2 * b]["out"].astype(np.float32)
                  + res.results[2 * b + 1]["out"].astype(np.float32))
    return out


# revision 18
# speedup vs baseline: 1.0199x; 1.0199x over previous
"""MultiHeadAttention (B=4,T=2048,D=2048,NQ=16,NK=8,H=128) on 8 trn2 cores.

Sharding: core c -> batch b=c//2, half=c%2. Each core computes the partial
output for batch b restricted to q-heads [half*8, half*8+8) (kv-heads
[half*4, half*4+4)); host sums the two partials per batch (o_proj
contraction over heads is split across the core pair).

v3: V projection computed with 512-col matmuls (4 kv heads batched in the
free dim) eliminating 768 tiny matmuls; o_proj runs in fp8e4 DoubleRow
(2 heads' contraction per pass); RMSNorm gain broadcast moved to gpsimd;
o_proj psum->sbuf staging moved to gpsimd with the fp8 weight prescale
folded in; startup DMA order tuned (per-pair q weights, per-chunk rope
tables) so the first matmul issues ~8us in.
"""
import numpy as np
import concourse.bass as bass
import concourse.tile as tile
from concourse import bacc, mybir
from concourse import bass_utils

B, T, D = 4, 2048, 2048
NQ, NK, H = 16, 8, 128
NH, NKV = 8, 4          # per-core q heads / kv heads
THETA = 10000.0
EPS = 1e-6
TCH = 512               # chunk of T for projections / attention q blocks
NCH = T // TCH
NDK = D // 128
NQC = T // TCH
WO_SCALE = 256.0        # fp8 o_proj weight prescale

f32 = mybir.dt.float32
bf16 = mybir.dt.bfloat16
f8 = mybir.dt.float8e4
npbf16 = mybir.dt.np(bf16)
npf8 = mybir.dt.np(f8)
AF = mybir.ActivationFunctionType
DR = mybir.MatmulPerfMode.DoubleRow

TRACE = False
LAST_EXEC_NS = None
_CACHE = {}


def _install_hook():
    import contextlib, ctypes, sys, types
    if "antenv.axon_hooks" in sys.modules:
        return
    lib = ctypes.CDLL("/opt/axon/libaxon_pjrt.so")
    lib.axon_start_nrt_profile.argtypes = [ctypes.POINTER(ctypes.c_int64), ctypes.c_size_t]
    lib.axon_start_nrt_profile.restype = ctypes.c_int64
    lib.axon_stop_nrt_profile.argtypes = [ctypes.c_char_p]
    lib.axon_stop_nrt_profile.restype = ctypes.c_int64

    @contextlib.contextmanager
    def _hook(output_dir, device_ids):
        import jax
        jax.devices()
        ids = (ctypes.c_int64 * len(device_ids))(*device_ids) if device_ids else None
        rc = lib.axon_start_nrt_profile(ids, len(device_ids) if device_ids else 0)
        if rc != 0:
            raise RuntimeError(f"axon_start_nrt_profile rc={rc}")
        try:
            yield
        finally:
            n = lib.axon_stop_nrt_profile(str(output_dir).encode())
            if n < 0:
                raise RuntimeError(f"axon_stop_nrt_profile rc={n}")

    mod = types.ModuleType("antenv.axon_hooks")
    mod.get_axon_ntff_profile_hook = lambda: _hook
    mod.set_axon_ntff_profile_hook = lambda h: None
    sys.modules["antenv.axon_hooks"] = mod
    bass_utils.upload_artifacts = lambda tmpdir: "local://" + str(tmpdir)


def _build():
    nc = bacc.Bacc("TRN2", target_bir_lowering=False, debug=False, num_devices=8)
    xt_ap = nc.dram_tensor("xt", [D, T], bf16, kind="ExternalInput").ap()
    wq_aps = [nc.dram_tensor(f"wq{p}", [128, 2 * NDK * 128], bf16,
                             kind="ExternalInput").ap() for p in range(4)]
    wk_ap = nc.dram_tensor("wk", [128, NKV * NDK * 128], bf16, kind="ExternalInput").ap()
    wv_ap = nc.dram_tensor("wv", [128, NDK * NKV * 128], bf16, kind="ExternalInput").ap()
    wo_ap = nc.dram_tensor("wo", [128, NH * D], f8, kind="ExternalInput").ap()
    cs_ap = nc.dram_tensor("cs", [128, T], bf16, kind="ExternalInput").ap()
    sn_ap = nc.dram_tensor("sn", [128, T], bf16, kind="ExternalInput").ap()
    qsc_ap = nc.dram_tensor("qsc", [128, 1], f32, kind="ExternalInput").ap()
    ksc_ap = nc.dram_tensor("ksc", [128, 1], f32, kind="ExternalInput").ap()
    cm_ap = nc.dram_tensor("cm", [128, 128], bf16, kind="ExternalInput").ap()
    rm_ap = nc.dram_tensor("rmat", [128, 128], bf16, kind="ExternalInput").ap()
    out_ap = nc.dram_tensor("out", [T, D], bf16, kind="ExternalOutput").ap()

    with tile.TileContext(nc) as tc:
        with tc.tile_pool(name="mp", bufs=1) as mp, \
             tc.tile_pool(name="pp", bufs=1, space="PSUM") as pp:
            # ---- persistent tiles; DMA queue order is startup-critical ----
            qsc_t = mp.tile([128, 1], f32)
            nc.sync.dma_start(qsc_t[:], qsc_ap[:])
            ksc_t = mp.tile([128, 1], f32)
            nc.sync.dma_start(ksc_t[:], ksc_ap[:])
            cm_t = mp.tile([128, 128], bf16)
            nc.sync.dma_start(cm_t[:], cm_ap[:])
            rmat_t = mp.tile([128, 128], bf16)
            nc.sync.dma_start(rmat_t[:], rm_ap[:])

            wq_ts = []
            for p in range(4):
                wt = mp.tile([128, 2 * NDK * 128], bf16, name=f"wq{p}")
                wq_ts.append(wt)
            xt_r = xt_ap.rearrange("(a p) t -> p a t", p=128)

            def load_xh(pool, ch):
                t0 = ch * TCH
                xh0 = pool.tile([128, 8 * TCH], bf16, tag="xh0", bufs=2, name="xh0")
                nc.sync.dma_start(
                    xh0[:].rearrange("p (a t) -> p a t", a=8),
                    xt_r[:, 0:8, t0:t0 + TCH])
                xh1 = pool.tile([128, 8 * TCH], bf16, tag="xh1", bufs=2, name="xh1")
                nc.sync.dma_start(
                    xh1[:].rearrange("p (a t) -> p a t", a=8),
                    xt_r[:, 8:16, t0:t0 + TCH])
                return xh0, xh1

            # q weights for heads 0-1 first, then x chunk 0, then the rest.
            nc.sync.dma_start(wq_ts[0][:], wq_aps[0][:])

            eps_t = mp.tile([1, 1], f32)
            nc.vector.memset(eps_t[:], EPS)
            ones_col_b = mp.tile([128, 1], bf16)
            nc.vector.memset(ones_col_b[:], 1.0)

            qT = mp.tile([128, NH * T], bf16)     # 32KB/part
            kT = mp.tile([128, NKV * T], bf16)    # 16KB
            vT = mp.tile([128, NKV * T], bf16)    # 16KB; cols = btile*512+kv*128+h

            rows = pp.tile([128, 512], f32, tag="a7")

            cs_ts, sn_ts = [], []
            wo_t = mp.tile([128, NH * D], f8)

            def drain_group(pool, accs, sc_t, ch, dsts):
                """accs: psum [128,512] f32 -> RMSNorm*(gain) + RoPE -> dsts bf16."""
                n = len(accs)
                for i in range(n):
                    sq = pool.tile([128, TCH], bf16, tag="sq", bufs=2, name=f"sq{i}")
                    nc.scalar.activation(sq[:], accs[i], AF.Square)
                    row = pp.tile([1, 512], f32, tag=f"a{4 + i % 2}", name=f"row{i}")
                    nc.tensor.matmul(row[:], ones_col_b[:], sq[:], start=True, stop=True)
                    rinv = pool.tile([1, TCH], f32, tag="rinv", bufs=2, name=f"rinv{i}")
                    nc.vector.reciprocal_approx_fast(rinv[:], row[:])
                    rstd = pool.tile([1, TCH], bf16, tag="rstd", bufs=2, name=f"rstd{i}")
                    nc.scalar.activation(rstd[:], rinv[:], AF.Sqrt)
                    bc = pool.tile([128, TCH], bf16, tag="bc", bufs=2, name=f"bc{i}")
                    nc.gpsimd.partition_broadcast(bc[:], rstd[:])
                    # qn0 = acc * gain (per-partition); rstd applied after rope
                    # (valid: rstd is per-column, invariant under the half-swap)
                    qn = pool.tile([128, TCH], bf16, tag="qn", bufs=2, name=f"qn{i}")
                    nc.scalar.activation(qn[:], accs[i], AF.Copy, scale=sc_t[:])
                    # RoPE half-swap via SBUF->SBUF DMA (scalar queue): frees
                    # the PE matmul + makes the sin-mul an all-bf16 2x op
                    qsw = pool.tile([128, TCH], bf16, tag="qsw", bufs=2, name=f"qsw{i}")
                    nc.scalar.dma_start(qsw[0:64, :], qn[64:128, :])
                    nc.scalar.dma_start(qsw[64:128, :], qn[0:64, :])
                    ta = pool.tile([128, TCH], bf16, tag="ta", bufs=2, name=f"ta{i}")
                    nc.vector.tensor_mul(ta[:], qn[:], cs_ts[ch][:])
                    tb = pool.tile([128, TCH], bf16, tag="tb", bufs=2, name=f"tb{i}")
                    nc.vector.tensor_mul(tb[:], qsw[:], sn_ts[ch][:])
                    rs = pool.tile([128, TCH], bf16, tag="rs", bufs=2, name=f"rs{i}")
                    nc.vector.tensor_add(rs[:], ta[:], tb[:])
                    nc.vector.tensor_mul(dsts[i], rs[:], bc[:])

            # ---- phase 1: q/k/v projection, norm+rope, all SBUF-resident ----
            with tc.tile_pool(name="pj", bufs=1) as pj:
                acc_roll = [0]

                def chain(w_t, whead, xh0, xh1):
                    """Sequential 16-dk matmul chain into one rolling psum bank."""
                    acc = pp.tile([128, 512], f32, tag=f"a{acc_roll[0] % 4}", name="acc")
                    acc_roll[0] += 1
                    for dk in range(NDK):
                        xh = xh0 if dk < 8 else xh1
                        nc.tensor.matmul(
                            acc[:],
                            w_t[:, (whead * NDK + dk) * 128:(whead * NDK + dk + 1) * 128],
                            xh[:, (dk % 8) * TCH:(dk % 8 + 1) * TCH],
                            start=(dk == 0), stop=(dk == NDK - 1))
                    return acc

                xhs = load_xh(pj, 0)
                # remaining weight/table loads (sync queue, after chunk-0 x)
                for p in range(1, 4):
                    nc.sync.dma_start(wq_ts[p][:], wq_aps[p][:])
                wk_t = pj.tile([128, NKV * NDK * 128], bf16)
                for j in range(2):
                    s = NKV * NDK * 128 // 2
                    nc.sync.dma_start(wk_t[:, j * s:(j + 1) * s], wk_ap[:, j * s:(j + 1) * s])
                wv_t = pj.tile([128, NDK * NKV * 128], bf16)
                for j in range(2):
                    s = NDK * NKV * 128 // 2
                    nc.sync.dma_start(wv_t[:, j * s:(j + 1) * s], wv_ap[:, j * s:(j + 1) * s])
                for ch in range(NCH):
                    t0 = ch * TCH
                    cst = mp.tile([128, TCH], bf16, name=f"cs{ch}")
                    nc.sync.dma_start(cst[:], cs_ap[:, t0:t0 + TCH])
                    snt = mp.tile([128, TCH], bf16, name=f"sn{ch}")
                    nc.sync.dma_start(snt[:], sn_ap[:, t0:t0 + TCH])
                    cs_ts.append(cst)
                    sn_ts.append(snt)
                for ch in range(NCH):
                    t0 = ch * TCH
                    xh0, xh1 = xhs
                    # prefetch next chunk's x before this chunk's drains/V
                    if ch + 1 < NCH:
                        xhs = load_xh(pj, ch + 1)
                    # q heads, two groups of 4
                    for g in range(2):
                        accs = [chain(wq_ts[(g * 4 + i) // 2], (g * 4 + i) % 2, xh0, xh1)
                                for i in range(4)]
                        drain_group(pj, [a[:] for a in accs], qsc_t, ch,
                                    [qT[:, (g * 4 + i) * T + t0:(g * 4 + i) * T + t0 + TCH]
                                     for i in range(4)])
                    # k heads
                    accs = [chain(wk_t, i, xh0, xh1) for i in range(NKV)]
                    drain_group(pj, [a[:] for a in accs], ksc_t, ch,
                                [kT[:, kv * T + t0:kv * T + t0 + TCH] for kv in range(NKV)])
                    # v: [t, (kv h)] orientation via 512-col matmuls
                    # (lhsT = x t-block, rhs = all 4 kv heads' w columns)
                    for tb_ in range(4):
                        vps = pp.tile([128, 512], f32, tag=f"a{acc_roll[0] % 4}", name="vps")
                        acc_roll[0] += 1
                        for dk in range(NDK):
                            xh = xh0 if dk < 8 else xh1
                            nc.tensor.matmul(
                                vps[:],
                                xh[:, (dk % 8) * TCH + tb_ * 128:(dk % 8) * TCH + (tb_ + 1) * 128],
                                wv_t[:, dk * 512:(dk + 1) * 512],
                                start=(dk == 0), stop=(dk == NDK - 1),
                                skip_group_check=True)
                        nc.vector.tensor_copy(
                            vT[:, (ch * 4 + tb_) * 512:(ch * 4 + tb_ + 1) * 512], vps[:])
                    if ch == 0:
                        # o_proj weights after chunk-0 traffic (needed in phase 2)
                        for j in range(4):
                            s = NH * D // 4
                            nc.sync.dma_start(wo_t[:, j * s:(j + 1) * s],
                                              wo_ap[:, j * s:(j + 1) * s])

            # ---- phase 2: attention + fused o_proj (fp8 DoubleRow) ----
            # o_proj units of chunk qi-1 are interleaved between attention
            # heads of chunk qi: their matmuls fill the PE stalls where PV
            # waits on exp, and they use no table-based ACT functions.
            with tc.tile_pool(name="op", bufs=1) as op:
                wo_r = wo_t[:].rearrange("p (h d) -> p h d", h=NH)
                out_r = out_ap.rearrange("(a p) d -> p a d", p=128)
                pend = []

                def mk_head(qi, h, attn_sb, nkj, q0):
                    def f():
                        kv = h // 2
                        o_ps = pp.tile([128, 512], f32, tag=f"a{5 + h % 2}", name="ops")
                        acc_sb = op.tile([128, TCH], bf16, tag="accsb", bufs=3, name="accsb")

                        def emit_s(kj):
                            m = kj - 4 * qi
                            lo = 128 * m if m > 0 else 0
                            s_ps = pp.tile([128, 512], f32, tag=f"a{kj % 4}", name="sps")
                            nc.tensor.matmul(
                                s_ps[:, lo:512],
                                kT[:, kv * T + kj * 128:kv * T + (kj + 1) * 128],
                                qT[:, h * T + q0 + lo:h * T + q0 + TCH],
                                start=True, stop=True)
                            pt = op.tile([128, TCH], bf16, tag="pt", bufs=6, name="pt")
                            return s_ps, pt, lo, m, kj

                        def emit_drain(s_ps, pt, lo, m, kj):
                            nc.scalar.activation(pt[:, lo:512], s_ps[:, lo:512], AF.Exp)
                            if m >= 0:
                                nc.vector.tensor_mul(pt[:, lo:lo + 128],
                                                     pt[:, lo:lo + 128], cm_t[:])
                            if kj == 0:
                                nc.vector.tensor_copy(acc_sb[:], pt[:])
                            else:
                                nc.vector.tensor_add(acc_sb[:, lo:512],
                                                     acc_sb[:, lo:512], pt[:, lo:512])
                            nc.tensor.matmul(
                                o_ps[:, lo:512],
                                vT[:, kj * 512 + kv * 128:kj * 512 + (kv + 1) * 128],
                                pt[:, lo:512],
                                start=(kj == 0), stop=(kj == nkj - 1),
                                skip_group_check=True)

                        prev = None
                        for kj in range(nkj):
                            cur = emit_s(kj)
                            if prev is not None:
                                emit_drain(*prev)
                            prev = cur
                        emit_drain(*prev)

                        # softmax denominator for this (h, qi).
                        # NB: reciprocal_approx_fast corrupts results when its
                        # input sits at a nonzero base partition -> keep row 0.
                        nc.tensor.matmul(rows[0:1, :], ones_col_b[:],
                                         acc_sb[:], start=True, stop=True)
                        rden = op.tile([1, TCH], f32, tag="rden", bufs=3, name="rden")
                        nc.vector.reciprocal_approx_fast(rden[:], rows[0:1, :])
                        rbc = op.tile([128, TCH], f32, tag="rbc", bufs=3, name="rbc")
                        nc.gpsimd.partition_broadcast(rbc[:], rden[:])
                        nc.vector.tensor_mul(
                            attn_sb[:, h * TCH:(h + 1) * TCH], o_ps[:], rbc[:])
                    return f

                def mk_oproj(qi, dc, attn_r):
                    def f():
                        stg4 = op.tile([128, 4 * 512], bf16, tag="ostg", bufs=2, name="ostg")
                        for ti in range(4):
                            ops2 = pp.tile([128, 512], f32, tag="a4", name="ops2")
                            for g in range(4):
                                nc.tensor.matmul(
                                    ops2[:],
                                    attn_r[:, 2 * g:2 * g + 2, ti * 128:(ti + 1) * 128],
                                    wo_r[:, 2 * g:2 * g + 2, dc * 512:(dc + 1) * 512],
                                    start=(g == 0), stop=(g == 3),
                                    perf_mode=DR)
                            # psum -> sbuf with the fp8 weight prescale undone
                            if ti % 2 == 0:
                                nc.vector.tensor_scalar_mul(
                                    stg4[:, ti * 512:(ti + 1) * 512], ops2[:],
                                    1.0 / WO_SCALE)
                            else:
                                nc.scalar.activation(stg4[:, ti * 512:(ti + 1) * 512],
                                                     ops2[:], AF.Copy, scale=1.0 / WO_SCALE)
                        nc.sync.dma_start(
                            out_r[:, qi * 4:qi * 4 + 4, dc * 512:(dc + 1) * 512],
                            stg4[:].rearrange("p (a d) -> p a d", a=4))
                    return f

                for qi in range(NQC):
                    q0 = qi * TCH
                    attn_sb = op.tile([128, NH * TCH], f8, tag="attn", bufs=2, name="attn")
                    attn_r = attn_sb[:].rearrange("p (h t) -> p h t", h=NH)
                    nkj = 4 * qi + 4
                    for h in range(NH):
                        mk_head(qi, h, attn_sb, nkj, q0)()
                        if pend and h % 2 == 1:
                            pend.pop(0)()
                    pend = [mk_oproj(qi, dc, attn_r) for dc in range(4)]
                for u in pend:
                    u()

    nc.compile()
    return nc


def _pack(w):
    """(nh, D, H) -> (128, nh*NDK*128): col block (h*NDK+dk)*128 = w[h, dk*128:+128, :]."""
    nh = w.shape[0]
    a = w.reshape(nh, NDK, 128, H).transpose(2, 0, 1, 3)
    return np.ascontiguousarray(a.reshape(128, nh * NDK * H)).astype(npbf16)


def _pack_v(w):
    """(NKV, D, H) -> (128, NDK*NKV*128): col dk*512+kv*128+h = w[kv, dk*128+p, h]."""
    a = w.reshape(NKV, NDK, 128, H).transpose(2, 1, 0, 3)
    return np.ascontiguousarray(a.reshape(128, NDK * NKV * H)).astype(npbf16)


def _numpy_ref(x, mask, position, qp, kvp, op, qns, kns):
    def rms(v, s):
        var = (v * v).mean(-1, keepdims=True)
        return v / np.sqrt(var + EPS) * (1.0 + s)

    def rope(v, pos):
        ts = THETA ** (np.arange(64, dtype=np.float32) * 2.0 / H)
        ang = pos.astype(np.float32)[:, :, None, None] / ts
        sn, cs = np.sin(ang), np.cos(ang)
        x1, x2 = v[..., :64], v[..., 64:]
        return np.concatenate([x1 * cs - x2 * sn, x2 * cs + x1 * sn], -1)

    q = np.einsum('BTD,NDH->BTNH', x, qp)
    k = np.einsum('BTD,KDH->BTKH', x, kvp[0])
    v = np.einsum('BTD,KDH->BTKH', x, kvp[1])
    q = rope(rms(q, qns), position) * (H ** -0.5)
    k = rope(rms(k, kns), position)
    q = q.transpose(0, 2, 1, 3)
    k = np.repeat(k.transpose(0, 2, 1, 3), NQ // NK, 1)
    v = np.repeat(v.transpose(0, 2, 1, 3), NQ // NK, 1)
    s = np.einsum('BHtD,BHTD->BHtT', q, k) / np.sqrt(np.float32(H))
    s = np.where(mask[:, None], s, np.float32(-2.3819763e+38))
    s = s - s.max(-1, keepdims=True)
    w = np.exp(s)
    w /= w.sum(-1, keepdims=True)
    o = np.einsum('BHtT,BHTD->BHtD', w, v)
    return np.einsum('BNTH,NHD->BTD', o, op).astype(np.float32)


def kernel(**inputs):
    global LAST_EXEC_NS
    x = np.asarray(inputs["x"], np.float32)
    mask = np.asarray(inputs["mask"])
    position = np.asarray(inputs["position"])
    qp = np.asarray(inputs["q_proj"], np.float32)
    kvp = np.asarray(inputs["kv_proj"], np.float32)
    op = np.asarray(inputs["o_proj"], np.float32)
    qns = np.asarray(inputs["q_norm_scale"], np.float32)
    kns = np.asarray(inputs["k_norm_scale"], np.float32)

    tril = np.tril(np.ones((T, T), bool))
    if mask.shape != (B, T, T) or not all(np.array_equal(mask[b], tril) for b in range(B)):
        return _numpy_ref(x, mask, position, qp, kvp, op, qns, kns)

    if "nc" not in _CACHE:
        _CACHE["nc"] = _build()
    nc = _CACHE["nc"]

    halves = []
    for half in range(2):
        wq_full = _pack(qp[half * NH:(half + 1) * NH])
        wqs = [np.ascontiguousarray(wq_full[:, p * 2 * NDK * 128:(p + 1) * 2 * NDK * 128])
               for p in range(4)]
        halves.append((
            wqs,
            _pack(kvp[0, half * NKV:(half + 1) * NKV]),
            _pack_v(kvp[1, half * NKV:(half + 1) * NKV]),
            np.ascontiguousarray(
                (op[half * NH:(half + 1) * NH] * WO_SCALE)
                .transpose(1, 0, 2).reshape(128, NH * D)
            ).astype(npf8),
        ))
    qsc = ((1.0 + qns) / np.sqrt(H)).reshape(128, 1).astype(np.float32)
    ksc = ((1.0 + kns) * np.sqrt(H)).reshape(128, 1).astype(np.float32)
    ts = THETA ** (np.arange(64, dtype=np.float64) * 2.0 / H)
    pidx = np.arange(128)[:, None]
    fidx = np.arange(128)[None, :]
    cm = (fidx >= pidx).astype(npbf16)
    rmat = np.zeros((128, 128), np.float32)
    rmat[(np.arange(128) + 64) % 128, np.arange(128)] = 1.0
    rmat = rmat.astype(npbf16)

    in_maps = []
    for c in range(8):
        b, half = c // 2, c % 2
        wqs, wk, wv, wo = halves[half]
        ang = position[b].astype(np.float64)[None, :] / ts[:, None]
        sn = np.sin(ang).astype(np.float32)
        cs = np.cos(ang).astype(np.float32)
        im = {
            "xt": np.ascontiguousarray(x[b].T).astype(npbf16),
            "wk": wk, "wv": wv, "wo": wo,
            "cs": np.ascontiguousarray(np.concatenate([cs, cs], 0)).astype(npbf16),
            "sn": np.ascontiguousarray(np.concatenate([-sn, sn], 0)).astype(npbf16),
            "qsc": qsc, "ksc": ksc, "cm": cm, "rmat": rmat,
        }
        for p in range(4):
            im[f"wq{p}"] = wqs[p]
        in_maps.append(im)

    if TRACE:
        _install_hook()
    last_err = None
    for _ in range(3):
        try:
            res = bass_utils.run_bass_kernel_spmd(nc, in_maps, list(range(8)), trace=TRACE)
            break
        except Exception as e:  # transient NRT device wedge
            last_err = e
    else:
        raise last_err
    LAST_EXEC_NS = getattr(res, "exec_time_ns", None)

    out = np.empty((B, T, D), np.float32)
    for b in range(B):
        out[b] = (res.results[2 * b]["out"].astype(np.float32)
                  + res.results[2 * b + 1]["out"].astype(np.float32))
    return out
